# revision 1
# baseline (speedup 1.0000x reference)
"""Trainium2 Bass kernel for local (neighbor-list) multi-head attention.

Sharding: 8 cores = 2 frames x 4 atom-chunks (512 local atoms per core).
Per core: project k rows to SBUF (rank-striped) and v rows to DRAM in
fp16, DMA-row-gather neighbors (dma_gather; K from SBUF via transpose
mode, V from HBM), per-block batched QK (M=32 block-diag stationaries),
softmax over a host-masked full-width bias, PE-transpose, per-atom AV
(tiny 8-col stationaries), diagonal extraction via a DRAM bounce,
gating + output projection.
"""

import numpy as np

NF, NLOC, NALL, NNEI = 2, 2048, 3072, 128
H, D = 8, 32
TOTAL = H * D          # 256
QDIM = 256
NCORES = 8
CPF = NCORES // NF     # 4 cores per frame
NLOC_C = NLOC // CPF   # 512 atoms per core
BLK = 16               # atoms per block
NBLK = NLOC_C // BLK   # 32
SG = 4                 # blocks per AV-extract supergroup
NSG = NBLK // SG       # 8

_CACHE = {}


def _build():
    import concourse.bass as bass
    import concourse.mybir as mybir
    from concourse import bacc
    from concourse.tile import TileContext
    from concourse.masks import make_identity

    dt = mybir.dt
    f32, f16, i16 = dt.float32, dt.float16, dt.int16
    AF = mybir.ActivationFunctionType

    nc = bacc.Bacc(None, target_bir_lowering=False)

    # ---------------- external inputs (contents differ per core) ------------
    qT = nc.dram_tensor("qT", [QDIM, NLOC_C], f16, kind="ExternalInput")
    kT = nc.dram_tensor("kT", [QDIM, NALL], f16, kind="ExternalInput")
    vT = nc.dram_tensor("vT", [QDIM, NALL], f16, kind="ExternalInput")
    WqT = nc.dram_tensor("WqT", [QDIM, TOTAL], f16, kind="ExternalInput")
    WgT = nc.dram_tensor("WgT", [QDIM, TOTAL], f16, kind="ExternalInput")
    WkT = nc.dram_tensor("WkT", [QDIM, TOTAL], f16, kind="ExternalInput")
    WvT = nc.dram_tensor("WvT", [QDIM, TOTAL], f16, kind="ExternalInput")
    WoTh = nc.dram_tensor("WoTh", [TOTAL, QDIM], f16, kind="ExternalInput")
    bgr = nc.dram_tensor("bgr", [128, TOTAL], f32, kind="ExternalInput")
    bo2 = nc.dram_tensor("bo2", [128, 2], f32, kind="ExternalInput")
    idx = nc.dram_tensor("idx", [128, NBLK * NNEI], i16, kind="ExternalInput")
    bias_p = nc.dram_tensor("bias_p", [NBLK, 128, 4 * NNEI], f16, kind="ExternalInput")

    out_t = nc.dram_tensor("out_t", [TOTAL, NLOC_C], f32, kind="ExternalOutput")

    with TileContext(nc) as tc:
        with (
            tc.tile_pool(name="const", bufs=1) as const,
            tc.tile_pool(name="work", bufs=2) as work,
            tc.tile_pool(name="gath", bufs=2) as gath,
            tc.tile_pool(name="psA", bufs=2, space="PSUM") as psA,
            tc.tile_pool(name="psB", bufs=3, space="PSUM") as psB,
            tc.tile_pool(name="psC", bufs=3, space="PSUM") as psC,
            tc.tile_pool(name="dram", bufs=1, space="DRAM") as dram,
        ):
            # ---------------- constants & inputs to SBUF --------------------
            ident = const.tile([128, 128], f16, tag="ident")
            make_identity(nc, ident)

            wq = const.tile([128, 2, TOTAL], f16, tag="wq")
            nc.sync.dma_start(wq, WqT.rearrange("(a p) o -> p a o", p=128))
            wg = const.tile([128, 2, TOTAL], f16, tag="wg")
            nc.sync.dma_start(wg, WgT.rearrange("(a p) o -> p a o", p=128))
            wk = const.tile([128, 2, TOTAL], f16, tag="wk")
            nc.sync.dma_start(wk, WkT.rearrange("(a p) o -> p a o", p=128))
            wv = const.tile([128, 2, TOTAL], f16, tag="wv")
            nc.sync.dma_start(wv, WvT.rearrange("(a p) o -> p a o", p=128))
            wo = const.tile([128, 2, QDIM], f16, tag="wo")
            nc.sync.dma_start(wo, WoTh.rearrange("(a p) o -> p a o", p=128))
            bg_t = const.tile([128, TOTAL], f32, tag="bg_t")
            nc.sync.dma_start(bg_t, bgr[:, :])
            bo_t = const.tile([128, 2], f32, tag="bo_t")
            nc.sync.dma_start(bo_t, bo2[:, :])
            idx_t = const.tile([128, NBLK * NNEI], i16, tag="idx_t")
            nc.sync.dma_start(idx_t, idx[:, :])
            qT_t = const.tile([128, 2, NLOC_C], f16, tag="qT_t")
            nc.sync.dma_start(qT_t, qT.rearrange("(a p) n -> p a n", p=128))
            kT_t = const.tile([128, 2, NALL], f16, tag="kT_t")
            nc.sync.dma_start(kT_t, kT.rearrange("(a p) n -> p a n", p=128))
            vT_t = const.tile([128, 2, NALL], f16, tag="vT_t")
            nc.sync.dma_start(vT_t, vT.rearrange("(a p) n -> p a n", p=128))

            # ---------------- projections -----------------------------------
            # qhT (fp16, [hd_chunk][128, NLOC_C])
            qhT = const.tile([128, 2, NLOC_C], f16, tag="qhT")
            for hc in range(2):
                ps = psC.tile([128, NLOC_C], f32, tag="misc", name="ps_qh")
                for cc in range(2):
                    nc.tensor.matmul(
                        ps, wq[:, cc, 128 * hc:128 * (hc + 1)], qT_t[:, cc, :],
                        start=(cc == 0), stop=(cc == 1),
                    )
                nc.scalar.copy(qhT[:, hc, :], ps)

            # sigmoid(g) rows: [n_chunk][128, 256]
            sig_g = const.tile([128, 4, TOTAL], f32, tag="sig_g")
            for ncnk in range(4):
                ps = psC.tile([128, TOTAL], f32, tag="misc", name="ps_g")
                for cc in range(2):
                    nc.tensor.matmul(
                        ps, qT_t[:, cc, 128 * ncnk:128 * (ncnk + 1)], wg[:, cc, :],
                        start=(cc == 0), stop=(cc == 1),
                    )
                gtmp = work.tile([128, TOTAL], f32, tag="gtmp")
                nc.vector.tensor_add(gtmp, ps, bg_t)
                nc.scalar.activation(sig_g[:, ncnk, :], gtmp, AF.Sigmoid)

            # qblk: block-diagonal stationaries [128, ch, NBLK*4 groups * 32]
            qblk = const.tile([128, 2, (NLOC_C // 4) * 32], f16, tag="qblk")
            nc.gpsimd.memset(qblk, 0.0)
            for ch in range(2):
                for qq in range(4):
                    h = 4 * ch + qq
                    dst = qblk[32 * qq:32 * (qq + 1), ch, :].rearrange(
                        "p (G c) -> p G c", c=32
                    )[:, :, 4 * h:4 * h + 4]
                    src = qhT[32 * qq:32 * (qq + 1), ch, :].rearrange(
                        "p (G a) -> p G a", a=4
                    )
                    nc.vector.tensor_copy(dst, src)

            # kh rows stay in SBUF (rank-striped for SBUF-source gather)
            khs = const.tile([128, NALL // 128, TOTAL], f16, tag="khs")
            for jc in range(NALL // 128):
                ps = psC.tile([128, TOTAL], f32, tag="misc", name="ps_k")
                for cc in range(2):
                    nc.tensor.matmul(
                        ps, kT_t[:, cc, 128 * jc:128 * (jc + 1)], wk[:, cc, :],
                        start=(cc == 0), stop=(cc == 1),
                    )
                nc.scalar.copy(khs[:, jc, :], ps)

            # vh rows -> DRAM fp16
            vh_d = dram.tile([NALL, TOTAL], f16)
            for jc4 in range(NALL // 512):
                row16 = work.tile([128, 4, TOTAL], f16, tag="row16")
                for j4 in range(4):
                    jc = 4 * jc4 + j4
                    ps = psC.tile([128, TOTAL], f32, tag="misc", name="ps_v")
                    for cc in range(2):
                        nc.tensor.matmul(
                            ps, vT_t[:, cc, 128 * jc:128 * (jc + 1)],
                            wv[:, cc, :],
                            start=(cc == 0), stop=(cc == 1),
                        )
                    nc.scalar.copy(row16[:, j4, :], ps)
                nc.sync.dma_start(
                    vh_d[512 * jc4:512 * (jc4 + 1), :].rearrange(
                        "(c p) o -> p c o", p=128
                    ),
                    row16,
                )

            # ---------------- main loop -------------------------------------
            o_scr = dram.tile([NLOC_C, TOTAL], f16)
            o_scr_r = o_scr.rearrange(
                "(sgx bv s) (hh d) -> sgx s bv hh d", bv=4 * SG, s=4, hh=H
            )
            stage = None
            kgT = vg = bias_sg = None
            for b in range(NBLK):
                sg, bi = b // SG, b % SG
                if b % 2 == 0:
                    idx_sl = idx_t[:, NNEI * b:NNEI * (b + 2)]
                    kgT = gath.tile([128, 2, 2 * BLK * NNEI], f16, tag="kgT")
                    nc.gpsimd.dma_gather(
                        kgT, khs[:, :, :], idx_sl,
                        num_idxs=2 * BLK * NNEI, num_idxs_reg=2 * BLK * NNEI,
                        elem_size=TOTAL, transpose=True, queue_num=0,
                        single_packet=False,
                        sbuf_tokens_per_rank=128,
                        sbuf_free_dim_per_rank=2 * TOTAL,
                        sbuf_free_dim_pad_per_rank=0,
                        sbuf_byte_offset=0,
                    )
                    vg = gath.tile([128, 2 * BLK, TOTAL], f16, tag="vg")
                    nc.gpsimd.dma_gather(
                        vg, vh_d[:, :], idx_sl,
                        num_idxs=2 * BLK * NNEI, num_idxs_reg=2 * BLK * NNEI,
                        elem_size=TOTAL, transpose=False, queue_num=0,
                        single_packet=False,
                    )
                if bi == 0:
                    bias_sg = work.tile([128, SG, 4 * NNEI], f16, tag="bias_sg", bufs=3)
                    nc.scalar.dma_start(
                        bias_sg,
                        bias_p[b:b + SG].rearrange("b p i -> p b i"),
                    )

                # QK: 4 groups x 2 chunks, M=32 (4 atoms x 8 heads blockdiag)
                qk = psA.tile([128, 4 * NNEI], f32, tag="qk")
                for g in range(4):
                    for ch in range(2):
                        nc.tensor.matmul(
                            qk[32 * g:32 * (g + 1), :],
                            qblk[:, ch, 32 * (4 * b + g):32 * (4 * b + g + 1)],
                            kgT[:, ch, 512 * (4 * (b % 2) + g):512 * (4 * (b % 2) + g + 1)],
                            start=(ch == 0), stop=(ch == 1),
                            tile_position=(0, 32 * g),
                        )

                # bias add (full width; garbage windows get -30000 from host)
                s_t = work.tile([128, 4 * NNEI], f32, tag="s_t", bufs=3)
                nc.vector.tensor_add(s_t, qk, bias_sg[:, bi, :])

                # softmax over free dim (garbage cols underflow to 0)
                m_t = work.tile([128, 1], f32, tag="m_t", bufs=3)
                nc.vector.reduce_max(
                    m_t, s_t, axis=mybir.AxisListType.X, negate=True
                )
                p_t = work.tile([128, 4 * NNEI], f16, tag="p_t", bufs=3)
                rsum = work.tile([128, 1], f32, tag="rsum", bufs=3)
                nc.scalar.activation(
                    p_t, s_t, AF.Exp, bias=m_t, scale=1.0, accum_out=rsum
                )
                rinv = work.tile([128, 1], f32, tag="rinv", bufs=3)
                nc.vector.reciprocal(rinv, rsum)
                pn = work.tile([128, 4 * NNEI], f16, tag="pn", bufs=3)
                nc.vector.tensor_scalar_mul(pn, p_t, rinv)

                # transpose each 128-col window -> pT[:, j, :]
                pT = work.tile([128, 4, 128], f16, tag="pT", bufs=3)
                for j in range(4):
                    pt_ps = psC.tile([128, 128], f16, tag="misc", name="pt_ps")
                    nc.tensor.transpose(
                        pt_ps, pn[:, 128 * j:128 * (j + 1)], ident
                    )
                    nc.vector.tensor_copy(pT[:, j, :], pt_ps)

                # AV: per-atom tiny stationaries
                av0 = psB.tile([128, 512], f32, tag="av", name="av0")
                av1 = psB.tile([128, 512], f32, tag="av", name="av1")
                avs = (av0, av1)
                for a in range(BLK):
                    s_, v_, bank = a % 4, (a // 4) % 2, a // 8
                    cb = 32 * (a // 4) + (a % 4)
                    nc.tensor.matmul(
                        avs[bank][32 * s_:32 * s_ + 8, 256 * v_:256 * (v_ + 1)],
                        pT[:, a % 4, cb:cb + 29:4],
                        vg[:, BLK * (b % 2) + a, :],
                        start=True, stop=True,
                        tile_position=(0, 32 * s_),
                    )

                # evac AV psum into supergroup staging
                if bi == 0:
                    stage = work.tile([128, SG * 1024], f16, tag="stage")
                nc.vector.tensor_copy(stage[:, 1024 * bi:1024 * bi + 512], av0)
                nc.scalar.copy(stage[:, 1024 * bi + 512:1024 * (bi + 1)], av1)

                if bi == SG - 1:
                    # diagonal extract: SBUF -> DRAM bounce (per head)
                    st_r = stage.rearrange(
                        "p (bv hh d) -> p bv hh d", hh=H, d=D
                    )
                    eng = (nc.sync, nc.scalar)
                    for h in range(H):
                        eng[h % 2].dma_start(
                            o_scr_r[sg, :, :, h, :],
                            st_r[h::32, :, h, :],
                        )

                if sg % 2 == 1 and bi == SG - 1:
                    # readback 128 atoms, gate, transpose, project
                    nck = sg // 2
                    base_n = nck * 128
                    orow = work.tile([128, TOTAL], f16, tag="orow")
                    nc.sync.dma_start(orow, o_scr[base_n:base_n + 128, :])
                    god = work.tile([128, TOTAL], f16, tag="god")
                    nc.vector.tensor_mul(god, orow, sig_g[:, nck, :])
                    godT = work.tile([128, 2, 128], f16, tag="godT")
                    for hc in range(2):
                        gps = psC.tile([128, 128], f16, tag="misc", name="gps")
                        nc.tensor.transpose(
                            gps, god[:, 128 * hc:128 * (hc + 1)], ident
                        )
                        nc.vector.tensor_copy(godT[:, hc, :], gps)
                    for oc in range(2):
                        ops = psC.tile([128, 128], f32, tag="misc", name="ops")
                        for hc in range(2):
                            nc.tensor.matmul(
                                ops, wo[:, hc, 128 * oc:128 * (oc + 1)],
                                godT[:, hc, :],
                                start=(hc == 0), stop=(hc == 1),
                            )
                        outs = work.tile([128, 128], f32, tag="outs")
                        nc.scalar.activation(
                            outs, ops, AF.Identity, bias=bo_t[:, oc:oc + 1]
                        )
                        nc.sync.dma_start(
                            out_t[128 * oc:128 * (oc + 1), base_n:base_n + 128],
                            outs,
                        )
    nc.finalize()
    return nc


def _host_prep(q, k, v, nlist, bias, Wq, Wk, Wv, Wg, bg, Wo, bo):
    """Build the 8 per-core input maps."""
    norm = D ** -0.5
    f32 = np.float32
    WqT = np.ascontiguousarray((Wq * norm).T.astype(np.float16))
    WgT = np.ascontiguousarray(Wg.T.astype(np.float16))
    WkT = np.ascontiguousarray(Wk.T.astype(np.float16))
    WvT = np.ascontiguousarray(Wv.T.astype(np.float16))
    WoTh = np.ascontiguousarray(Wo.T.astype(np.float16))
    bgr = np.ascontiguousarray(np.broadcast_to(bg.astype(f32), (128, TOTAL)))
    bo2 = np.ascontiguousarray(bo.astype(f32).reshape(2, 128).T)

    in_maps = []
    for c in range(NCORES):
        f, chunk = c // CPF, c % CPF
        n0 = chunk * NLOC_C
        qc = q[f, n0:n0 + NLOC_C]                     # [512, 256]
        nl = nlist[f, n0:n0 + NLOC_C].astype(np.int16)  # [512, 128]
        # wrapped gather indices: per block b, t-th index at [16g + t%16, t//16]
        w = nl.reshape(NBLK, BLK * NNEI).reshape(NBLK, BLK * NNEI // 16, 16)
        w = np.transpose(w, (0, 2, 1)).reshape(NBLK, 16, -1)   # [b, 16, 128]
        w = np.concatenate([w] * 8, axis=1)                    # [b, 128, 128]
        idx_full = np.ascontiguousarray(
            np.transpose(w, (1, 0, 2)).reshape(128, NBLK * NNEI)
        )
        # bias: [8, 512, 128] -> [32 blocks, (g h asub), 128]
        bs = bias[f, :, n0:n0 + NLOC_C, :]
        from einops import rearrange as rr
        bias_cmp = rr(bs, "h (b g asub) i -> b (g h asub) i", b=NBLK, g=4, asub=4)
        bias_c = np.full((NBLK, 128, 4 * NNEI), -30000.0, np.float16)
        p_arange = np.arange(128)
        for asub in range(4):
            rows = p_arange[p_arange % 4 == asub]
            bias_c[:, rows, NNEI * asub:NNEI * (asub + 1)] = (
                bias_cmp[:, rows, :].astype(np.float16)
            )
        in_maps.append({
            "qT": np.ascontiguousarray(qc.T.astype(np.float16)),
            "kT": np.ascontiguousarray(k[f].T.astype(np.float16)),
            "vT": np.ascontiguousarray(v[f].T.astype(np.float16)),
            "WqT": WqT, "WgT": WgT, "WkT": WkT, "WvT": WvT, "WoTh": WoTh,
            "bgr": bgr, "bo2": bo2,
            "idx": idx_full, "bias_p": bias_c,
        })
    return in_maps


def kernel(q, k, v, nlist, bias, Wq, Wk, Wv, Wg, bg, Wo, bo):
    from concourse.bass_utils import run_bass_kernel_spmd

    q = np.asarray(q, dtype=np.float32)
    k = np.asarray(k, dtype=np.float32)
    v = np.asarray(v, dtype=np.float32)
    bias = np.asarray(bias, dtype=np.float32)
    nlist_np = np.asarray(nlist)

    if "nc" not in _CACHE:
        _CACHE["nc"] = _build()
    nc = _CACHE["nc"]

    in_maps = _host_prep(
        q, k, v, nlist_np, bias,
        np.asarray(Wq, np.float32), np.asarray(Wk, np.float32),
        np.asarray(Wv, np.float32), np.asarray(Wg, np.float32),
        np.asarray(bg, np.float32), np.asarray(Wo, np.float32),
        np.asarray(bo, np.float32),
    )
    res = run_bass_kernel_spmd(nc, in_maps, core_ids=list(range(NCORES)))
    out = np.empty((NF, NLOC, TOTAL), dtype=np.float32)
    for c in range(NCORES):
        f, chunk = c // CPF, c % CPF
        n0 = chunk * NLOC_C
        out[f, n0:n0 + NLOC_C] = res.results[c]["out_t"].T
    return out

